# revision 9
# baseline (speedup 1.0000x reference)
"""Bahdanau-attention kernel for Trainium2, data-parallel over 8 NeuronCores.

V2: bf16 datapath with XBAR DMA-transposes — the PE runs only enc matmuls,
scores matmuls, and tiny K=1 replicates; context is a DVE fused
multiply-reduce. Host converts inputs to bf16 and un-transposes outputs.

Per core (B_local=8, T=1024, H=1024), per batch b:
  eT[o,t]   = tanh(sum_h W_enc[o,h] x[t,h] + (W_dec h + b_dec + b_enc)[o])
              (PE bf16 matmuls, xT/wT laid out by XBAR transpose-DMA;
               ScalarE tanh with per-partition bias)
  scores[t] = sum_o w_score[o] * eT[o,t]        (PE, ws chunk stationary)
  u         = exp(scores); w = u / sum(u)       (ScalarE exp+accum, DVE recip)
  u_rep     = ones^T w                          (PE K=1 matmul -> [128,T])
  ctxT[p,j] = sum_t xT[j][p,t] * u_rep[p,t]     (DVE tensor_tensor_reduce)
b_score dropped: softmax is shift-invariant so it cancels in both outputs.
Outputs: out_w [bl,T] rows; out_ctx [bl,P,NH] column-major (host transposes).
"""

import sys

if "/opt/trn_rl_repo" not in sys.path:
    sys.path.insert(0, "/opt/trn_rl_repo")

import numpy as np

B, T, H = 64, 1024, 1024
NCORES = 8
BL = B // NCORES  # batches per core
P = 128  # partitions
NH = H // P  # h chunks
NO = H // P  # o chunks
NS = 2  # free-dim halves of T
F = 512  # matmul free-dim slice (one PSUM bank of f32)

_CACHE = {}
LAST_RESULT = None


def build(bl=BL):
    import concourse.tile as tile
    from concourse import bacc, mybir

    f32 = mybir.dt.float32
    f32r = mybir.dt.float32r
    bf16 = mybir.dt.bfloat16
    AF = mybir.ActivationFunctionType
    OP = mybir.AluOpType
    AX = mybir.AxisListType

    nc = bacc.Bacc("TRN2", target_bir_lowering=False, debug=False, num_devices=NCORES)
    x_d = nc.declare_dram_parameter("spatial_feats", [bl, T, H], bf16, isOutput=False)
    hs_d = nc.declare_dram_parameter("hidden_state", [bl, H], bf16, isOutput=False)
    we_d = nc.declare_dram_parameter("W_enc", [H, H], bf16, isOutput=False)
    be_d = nc.declare_dram_parameter("b_enc", [H], f32, isOutput=False)
    wd_d = nc.declare_dram_parameter("W_dec", [H, H], bf16, isOutput=False)
    bd_d = nc.declare_dram_parameter("b_dec", [H], f32, isOutput=False)
    ws_d = nc.declare_dram_parameter("w_score", [H], bf16, isOutput=False)
    ctx_d = nc.declare_dram_parameter("out_ctx", [bl, P, NH], f32, isOutput=True)
    wout_d = nc.declare_dram_parameter("out_w", [bl, T], f32, isOutput=True)

    with tile.TileContext(nc) as tc:
        with (
            tc.tile_pool(name="const", bufs=1) as const,
            tc.tile_pool(name="wt", bufs=NH) as wt_pool,
            tc.tile_pool(name="wdt", bufs=NH) as wdt_pool,
            tc.tile_pool(name="xt", bufs=3 * NH) as xt_pool,
            tc.tile_pool(name="et", bufs=6) as et_pool,
            tc.tile_pool(name="urep", bufs=2) as urep_pool,
            tc.tile_pool(name="ctxsc", bufs=2) as ctxsc_pool,
            tc.tile_pool(name="ctxt", bufs=2) as ctxt_pool,
            tc.tile_pool(name="rows", bufs=4) as rows,
            tc.tile_pool(name="small", bufs=16) as small,
            tc.tile_pool(name="mmps", bufs=4, space="PSUM") as mm_ps,
            tc.tile_pool(name="scps", bufs=2, space="PSUM") as sc_ps,
            tc.tile_pool(name="urps", bufs=2, space="PSUM") as ur_ps,
        ):
            # ---- constants / small loads ----
            ones_f = const.tile([1, P], f32, tag="onesf")
            nc.gpsimd.memset(ones_f[:], 1.0)
            ones_col = const.tile([1, P], f32r, tag="ones")
            nc.vector.tensor_copy(ones_col[:], ones_f[:])

            be_sb = const.tile([P, NH], f32, tag="be")
            nc.gpsimd.dma_start(be_sb[:], be_d[:].rearrange("(c p) -> p c", p=P))
            bd_sb = const.tile([P, NH], f32, tag="bd")
            nc.gpsimd.dma_start(bd_sb[:], bd_d[:].rearrange("(c p) -> p c", p=P))
            bsum = const.tile([P, NH], f32, tag="bsum")
            nc.vector.tensor_add(bsum[:], be_sb[:], bd_sb[:])
            ws_sb = const.tile([P, NH], bf16, tag="ws")
            nc.gpsimd.dma_start(ws_sb[:], ws_d[:].rearrange("(c p) -> p c", p=P))
            hsT = const.tile([P, NH * bl], bf16, tag="hsT")
            for c in range(NH):
                nc.gpsimd.dma_start(
                    hsT[:, c * bl : (c + 1) * bl],
                    hs_d[:, c * P : (c + 1) * P].rearrange("b p -> p b"),
                )

            # ---- weight transposes via XBAR DMA (wdT first: the PE bias
            # matmuls are queued before enc and must not stall on late wdT) ----
            wdT = [
                wdt_pool.tile([P, H], bf16, tag="wdt", name=f"wdT_{j}")
                for j in range(NH)
            ]
            # NOTE: ALL XBAR transposes go through nc.sync — the X-bar is a
            # single shared unit; concurrent transposes from two DGE queues
            # interleave tiles and corrupt the output (observed on HW).
            for j in range(NH):
                nc.sync.dma_start_transpose(wdT[j][:], wd_d[:, j * P : (j + 1) * P])

            # ---- bias_all[o_off, ot*bl+b] = (W_dec h_b + b_dec + b_enc)[o] ----
            bias_all = const.tile([P, NO * bl], f32, tag="bias_all")
            for ot in range(NO):
                psd = mm_ps.tile([P, F], f32, tag="mmps")
                for j in range(NH):
                    nc.tensor.matmul(
                        psd[:, 0:bl],
                        wdT[j][:, ot * P : (ot + 1) * P],
                        hsT[:, j * bl : (j + 1) * bl],
                        start=(j == 0),
                        stop=(j == NH - 1),
                    )
                nc.vector.tensor_scalar_add(
                    bias_all[:, ot * bl : (ot + 1) * bl],
                    psd[:, 0:bl],
                    bsum[:, ot : ot + 1],
                )

            # ---- W_enc^T and x(0)^T via XBAR DMA, pairwise so enc h-chunk
            # matmuls can fire as each (wT[j], xT0[j]) pair lands ----
            wT = [
                wt_pool.tile([P, H], bf16, tag="wt", name=f"wT_{j}")
                for j in range(NH)
            ]
            xT0 = [
                xt_pool.tile([P, T], bf16, tag="xt", name=f"xT0_{j}")
                for j in range(NH)
            ]
            for j in range(NH):
                nc.sync.dma_start_transpose(wT[j][:], we_d[:, j * P : (j + 1) * P])
                nc.sync.dma_start_transpose(xT0[j][:], x_d[0, :, j * P : (j + 1) * P])

            def emit_enc_group(xT, it, o, s):
                ps = mm_ps.tile([P, F], f32, tag="mmps")
                for h in range(NH):
                    nc.tensor.matmul(
                        ps[:],
                        wT[h][:, o * P : (o + 1) * P],
                        xT[h][:, s * F : (s + 1) * F],
                        start=(h == 0),
                        stop=(h == NH - 1),
                    )
                e = et_pool.tile([P, F], bf16, tag="et")
                nc.scalar.activation(
                    e[:],
                    ps[:],
                    AF.Tanh,
                    bias=bias_all[:, o * bl + it : o * bl + it + 1],
                    scale=1.0,
                )
                return e

            def emit_finish(st):
                """Batch-tail work for batch st['it']: replicate normalized
                weights across partitions, fused multiply-reduce context,
                DMA both outputs."""
                it = st["it"]
                u_n = st["u_n"]
                urp = []
                for s in range(NS):
                    pu = ur_ps.tile([P, F], f32, tag="urps")
                    nc.tensor.matmul(
                        pu[:],
                        ones_col[:],
                        u_n[0:1, s * F : (s + 1) * F],
                        start=True,
                        stop=True,
                    )
                    urp.append(pu)
                u_rep = urep_pool.tile([P, T], bf16, tag="urep")
                for s in range(NS):
                    nc.scalar.activation(
                        u_rep[:, s * F : (s + 1) * F], urp[s][:], AF.Copy
                    )
                ctxT = ctxt_pool.tile([P, NH], f32, tag="ctxt")
                for j in range(NH):
                    scr = ctxsc_pool.tile([P, T], bf16, tag="ctxsc")
                    nc.vector.tensor_mul(scr[:], st["xT"][j][:], u_rep[:])
                    nc.vector.tensor_reduce(
                        ctxT[:, j : j + 1],
                        scr[:],
                        axis=AX.X,
                        op=OP.add,
                    )
                nc.sync.dma_start(ctx_d[it], ctxT[:])
                nc.sync.dma_start(wout_d[it : it + 1, :], u_n[:].bitcast(f32))

            # ---- main per-batch pipeline ----
            prev = None
            xT = xT0
            for it in range(bl):
                eT = {}
                pss = None
                xT_next = None
                for o in range(NO):
                    for s in range(NS):
                        eT[(o, s)] = emit_enc_group(xT, it, o, s)
                    if o == 0:
                        # After the first enc group: finish batch it-1 (its
                        # softmax chain has completed during this group), then
                        # prefetch x(it+1). Finish-first keeps the xT pool's
                        # reuse distance safe (readers before re-allocation).
                        if prev is not None:
                            emit_finish(prev)
                            prev = None
                        if it + 1 < bl:
                            xT_next = [
                                xt_pool.tile(
                                    [P, T], bf16, tag="xt", name=f"xT{it + 1}_{j}"
                                )
                                for j in range(NH)
                            ]
                            for j in range(NH):
                                nc.sync.dma_start_transpose(
                                    xT_next[j][:],
                                    x_d[it + 1, :, j * P : (j + 1) * P],
                                )
                        pss = {
                            s: sc_ps.tile([1, F], f32, tag="scps", name=f"pss{s}")
                            for s in range(NS)
                        }
                    else:
                        # scores run one o-chunk behind enc so the PE never
                        # waits on ScalarE's tanh
                        for s in range(NS):
                            nc.tensor.matmul(
                                pss[s][:],
                                ws_sb[:, o - 1 : o],
                                eT[(o - 1, s)][:],
                                start=(o - 1 == 0),
                                stop=False,
                            )
                # last scores group + softmax head (row layout)
                u_row = rows.tile([1, T], f32, tag="urow")
                accs = []
                for s in range(NS):
                    nc.tensor.matmul(
                        pss[s][:],
                        ws_sb[:, NO - 1 : NO],
                        eT[(NO - 1, s)][:],
                        start=False,
                        stop=True,
                    )
                    acc = small.tile([1, 1], f32, tag="acc")
                    nc.scalar.activation(
                        u_row[0:1, s * F : (s + 1) * F],
                        pss[s][:],
                        AF.Exp,
                        bias=0.0,
                        scale=1.0,
                        accum_out=acc[:],
                    )
                    accs.append(acc)
                ssum = small.tile([1, 1], f32, tag="ssum")
                nc.vector.tensor_add(ssum[:], accs[0][:], accs[1][:])
                rz = small.tile([1, 1], f32, tag="rz")
                nc.vector.reciprocal(rz[:], ssum[:])
                u_n = rows.tile([1, T], f32r, tag="un")
                nc.vector.tensor_scalar_mul(u_n[:], u_row[:], rz[0:1, 0:1])
                prev = {"xT": xT, "u_n": u_n, "it": it}
                if xT_next is not None:
                    xT = xT_next
            emit_finish(prev)

    nc.compile()
    return nc


def _get_nc(bl=BL):
    if bl not in _CACHE:
        _CACHE[bl] = build(bl)
    return _CACHE[bl]


def kernel(**inputs):
    import ml_dtypes
    from concourse.bass_utils import run_bass_kernel_spmd

    bf = ml_dtypes.bfloat16
    x = np.asarray(inputs["spatial_feats"], dtype=np.float32).astype(bf)
    hs = np.asarray(inputs["hidden_state"], dtype=np.float32).astype(bf)
    shared = {
        "W_enc": np.ascontiguousarray(
            np.asarray(inputs["W_enc"], dtype=np.float32).astype(bf)
        ),
        "W_dec": np.ascontiguousarray(
            np.asarray(inputs["W_dec"], dtype=np.float32).astype(bf)
        ),
        "w_score": np.ascontiguousarray(
            np.asarray(inputs["w_score"], dtype=np.float32).astype(bf)
        ),
        "b_enc": np.ascontiguousarray(np.asarray(inputs["b_enc"], dtype=np.float32)),
        "b_dec": np.ascontiguousarray(np.asarray(inputs["b_dec"], dtype=np.float32)),
    }
    nc = _get_nc()
    in_maps = []
    for i in range(NCORES):
        m = {
            "spatial_feats": np.ascontiguousarray(x[i * BL : (i + 1) * BL]),
            "hidden_state": np.ascontiguousarray(hs[i * BL : (i + 1) * BL]),
        }
        m.update(shared)
        in_maps.append(m)
    res = run_bass_kernel_spmd(nc, in_maps, core_ids=list(range(NCORES)))
    global LAST_RESULT
    LAST_RESULT = res
    ctx = np.concatenate(
        [
            res.results[i]["out_ctx"].transpose(0, 2, 1).reshape(BL, H)
            for i in range(NCORES)
        ],
        axis=0,
    )
    w = np.concatenate([res.results[i]["out_w"] for i in range(NCORES)], axis=0)
    return (ctx, w)


# revision 10
# speedup vs baseline: 1.1355x; 1.1355x over previous
"""Bahdanau-attention kernel for Trainium2, data-parallel over 8 NeuronCores.

V2: bf16 datapath with XBAR DMA-transposes — the PE runs only enc matmuls,
scores matmuls, and tiny K=1 replicates; context is a DVE fused
multiply-reduce. Host converts inputs to bf16 and un-transposes outputs.

Per core (B_local=8, T=1024, H=1024), per batch b:
  eT[o,t]   = tanh(sum_h W_enc[o,h] x[t,h] + (W_dec h + b_dec + b_enc)[o])
              (PE bf16 matmuls, xT/wT laid out by XBAR transpose-DMA;
               ScalarE tanh with per-partition bias)
  scores[t] = sum_o w_score[o] * eT[o,t]        (PE, ws chunk stationary)
  u         = exp(scores); w = u / sum(u)       (ScalarE exp+accum, DVE recip)
  u_rep     = ones^T w                          (PE K=1 matmul -> [128,T])
  ctxT[p,j] = sum_t xT[j][p,t] * u_rep[p,t]     (DVE tensor_tensor_reduce)
b_score dropped: softmax is shift-invariant so it cancels in both outputs.
Outputs: out_w [bl,T] rows; out_ctx [bl,P,NH] column-major (host transposes).
"""

import sys

if "/opt/trn_rl_repo" not in sys.path:
    sys.path.insert(0, "/opt/trn_rl_repo")

import numpy as np

B, T, H = 64, 1024, 1024
NCORES = 8
BL = B // NCORES  # batches per core
P = 128  # partitions
NH = H // P  # h chunks
NO = H // P  # o chunks
NS = 2  # free-dim halves of T
F = 512  # matmul free-dim slice (one PSUM bank of f32)

_CACHE = {}
LAST_RESULT = None


def build(bl=BL):
    import concourse.tile as tile
    from concourse import bacc, mybir

    f32 = mybir.dt.float32
    f32r = mybir.dt.float32r
    bf16 = mybir.dt.bfloat16
    AF = mybir.ActivationFunctionType
    OP = mybir.AluOpType
    AX = mybir.AxisListType

    nc = bacc.Bacc("TRN2", target_bir_lowering=False, debug=False, num_devices=NCORES)
    # spatial_feats / W_enc / W_dec are HOST-PRE-TRANSPOSED: x_d[b, h, t],
    # we_d[h, o], wd_d[h, o] — so every load is a plain contiguous DMA.
    x_d = nc.declare_dram_parameter("spatial_feats", [bl, H, T], bf16, isOutput=False)
    hs_d = nc.declare_dram_parameter("hidden_state", [bl, H], bf16, isOutput=False)
    we_d = nc.declare_dram_parameter("W_enc", [H, H], bf16, isOutput=False)
    be_d = nc.declare_dram_parameter("b_enc", [H], f32, isOutput=False)
    wd_d = nc.declare_dram_parameter("W_dec", [H, H], bf16, isOutput=False)
    bd_d = nc.declare_dram_parameter("b_dec", [H], f32, isOutput=False)
    ws_d = nc.declare_dram_parameter("w_score", [H], bf16, isOutput=False)
    ctx_d = nc.declare_dram_parameter("out_ctx", [bl, P, NH], f32, isOutput=True)
    wout_d = nc.declare_dram_parameter("out_w", [bl, T], f32, isOutput=True)

    with tile.TileContext(nc) as tc:
        with (
            tc.tile_pool(name="const", bufs=1) as const,
            tc.tile_pool(name="wt", bufs=NH) as wt_pool,
            tc.tile_pool(name="wdt", bufs=NH) as wdt_pool,
            tc.tile_pool(name="xt", bufs=3 * NH) as xt_pool,
            tc.tile_pool(name="et", bufs=6) as et_pool,
            tc.tile_pool(name="urep", bufs=2) as urep_pool,
            tc.tile_pool(name="ctxsc", bufs=2) as ctxsc_pool,
            tc.tile_pool(name="ctxt", bufs=2) as ctxt_pool,
            tc.tile_pool(name="rows", bufs=4) as rows,
            tc.tile_pool(name="small", bufs=16) as small,
            tc.tile_pool(name="mmps", bufs=4, space="PSUM") as mm_ps,
            tc.tile_pool(name="scps", bufs=2, space="PSUM") as sc_ps,
            tc.tile_pool(name="urps", bufs=2, space="PSUM") as ur_ps,
        ):
            # ---- constants / small loads ----
            ones_f = const.tile([1, P], f32, tag="onesf")
            nc.gpsimd.memset(ones_f[:], 1.0)
            ones_col = const.tile([1, P], f32r, tag="ones")
            nc.vector.tensor_copy(ones_col[:], ones_f[:])

            be_sb = const.tile([P, NH], f32, tag="be")
            nc.gpsimd.dma_start(be_sb[:], be_d[:].rearrange("(c p) -> p c", p=P))
            bd_sb = const.tile([P, NH], f32, tag="bd")
            nc.gpsimd.dma_start(bd_sb[:], bd_d[:].rearrange("(c p) -> p c", p=P))
            bsum = const.tile([P, NH], f32, tag="bsum")
            nc.vector.tensor_add(bsum[:], be_sb[:], bd_sb[:])
            ws_sb = const.tile([P, NH], bf16, tag="ws")
            nc.gpsimd.dma_start(ws_sb[:], ws_d[:].rearrange("(c p) -> p c", p=P))
            hsT = const.tile([P, NH * bl], bf16, tag="hsT")
            for c in range(NH):
                nc.gpsimd.dma_start(
                    hsT[:, c * bl : (c + 1) * bl],
                    hs_d[:, c * P : (c + 1) * P].rearrange("b p -> p b"),
                )

            # ---- weight transposes via XBAR DMA (wdT first: the PE bias
            # matmuls are queued before enc and must not stall on late wdT) ----
            wdT = [
                wdt_pool.tile([P, H], bf16, tag="wdt", name=f"wdT_{j}")
                for j in range(NH)
            ]
            for j in range(NH):
                eng = nc.sync if j % 2 == 0 else nc.scalar
                eng.dma_start(wdT[j][:], wd_d[j * P : (j + 1) * P, :])

            # ---- bias_all[o_off, ot*bl+b] = (W_dec h_b + b_dec + b_enc)[o] ----
            bias_all = const.tile([P, NO * bl], f32, tag="bias_all")
            for ot in range(NO):
                psd = mm_ps.tile([P, F], f32, tag="mmps")
                for j in range(NH):
                    nc.tensor.matmul(
                        psd[:, 0:bl],
                        wdT[j][:, ot * P : (ot + 1) * P],
                        hsT[:, j * bl : (j + 1) * bl],
                        start=(j == 0),
                        stop=(j == NH - 1),
                    )
                nc.vector.tensor_scalar_add(
                    bias_all[:, ot * bl : (ot + 1) * bl],
                    psd[:, 0:bl],
                    bsum[:, ot : ot + 1],
                )

            # ---- W_enc^T and x(0)^T via XBAR DMA, pairwise so enc h-chunk
            # matmuls can fire as each (wT[j], xT0[j]) pair lands ----
            wT = [
                wt_pool.tile([P, H], bf16, tag="wt", name=f"wT_{j}")
                for j in range(NH)
            ]
            xT0 = [
                xt_pool.tile([P, T], bf16, tag="xt", name=f"xT0_{j}")
                for j in range(NH)
            ]
            for j in range(NH):
                nc.sync.dma_start(wT[j][:], we_d[j * P : (j + 1) * P, :])
                eng = nc.scalar if j % 2 == 0 else nc.gpsimd
                eng.dma_start(xT0[j][:], x_d[0, j * P : (j + 1) * P, :])

            def emit_enc_group(xT, it, o, s):
                ps = mm_ps.tile([P, F], f32, tag="mmps")
                for h in range(NH):
                    nc.tensor.matmul(
                        ps[:],
                        wT[h][:, o * P : (o + 1) * P],
                        xT[h][:, s * F : (s + 1) * F],
                        start=(h == 0),
                        stop=(h == NH - 1),
                    )
                e = et_pool.tile([P, F], bf16, tag="et")
                nc.scalar.activation(
                    e[:],
                    ps[:],
                    AF.Tanh,
                    bias=bias_all[:, o * bl + it : o * bl + it + 1],
                    scale=1.0,
                )
                return e

            def emit_finish(st):
                """Batch-tail work for batch st['it']: replicate normalized
                weights across partitions, fused multiply-reduce context,
                DMA both outputs."""
                it = st["it"]
                u_n = st["u_n"]
                urp = []
                for s in range(NS):
                    pu = ur_ps.tile([P, F], f32, tag="urps")
                    nc.tensor.matmul(
                        pu[:],
                        ones_col[:],
                        u_n[0:1, s * F : (s + 1) * F],
                        start=True,
                        stop=True,
                    )
                    urp.append(pu)
                u_rep = urep_pool.tile([P, T], bf16, tag="urep")
                for s in range(NS):
                    nc.scalar.activation(
                        u_rep[:, s * F : (s + 1) * F], urp[s][:], AF.Copy
                    )
                ctxT = ctxt_pool.tile([P, NH], f32, tag="ctxt")
                for j in range(NH):
                    scr = ctxsc_pool.tile([P, T], bf16, tag="ctxsc")
                    nc.vector.tensor_mul(scr[:], st["xT"][j][:], u_rep[:])
                    nc.vector.tensor_reduce(
                        ctxT[:, j : j + 1],
                        scr[:],
                        axis=AX.X,
                        op=OP.add,
                    )
                nc.sync.dma_start(ctx_d[it], ctxT[:])
                nc.sync.dma_start(wout_d[it : it + 1, :], u_n[:].bitcast(f32))

            # ---- main per-batch pipeline ----
            prev = None
            xT = xT0
            for it in range(bl):
                eT = {}
                pss = None
                xT_next = None
                for o in range(NO):
                    for s in range(NS):
                        eT[(o, s)] = emit_enc_group(xT, it, o, s)
                    if o == 0:
                        # After the first enc group: finish batch it-1 (its
                        # softmax chain has completed during this group), then
                        # prefetch x(it+1). Finish-first keeps the xT pool's
                        # reuse distance safe (readers before re-allocation).
                        if prev is not None:
                            emit_finish(prev)
                            prev = None
                        if it + 1 < bl:
                            xT_next = [
                                xt_pool.tile(
                                    [P, T], bf16, tag="xt", name=f"xT{it + 1}_{j}"
                                )
                                for j in range(NH)
                            ]
                            for j in range(NH):
                                eng = nc.sync if j % 2 == 0 else nc.gpsimd
                                eng.dma_start(
                                    xT_next[j][:],
                                    x_d[it + 1, j * P : (j + 1) * P, :],
                                )
                        pss = {
                            s: sc_ps.tile([1, F], f32, tag="scps", name=f"pss{s}")
                            for s in range(NS)
                        }
                    else:
                        # scores run one o-chunk behind enc so the PE never
                        # waits on ScalarE's tanh
                        for s in range(NS):
                            nc.tensor.matmul(
                                pss[s][:],
                                ws_sb[:, o - 1 : o],
                                eT[(o - 1, s)][:],
                                start=(o - 1 == 0),
                                stop=False,
                            )
                # last scores group + softmax head (row layout)
                u_row = rows.tile([1, T], f32, tag="urow")
                accs = []
                for s in range(NS):
                    nc.tensor.matmul(
                        pss[s][:],
                        ws_sb[:, NO - 1 : NO],
                        eT[(NO - 1, s)][:],
                        start=False,
                        stop=True,
                    )
                    acc = small.tile([1, 1], f32, tag="acc")
                    nc.scalar.activation(
                        u_row[0:1, s * F : (s + 1) * F],
                        pss[s][:],
                        AF.Exp,
                        bias=0.0,
                        scale=1.0,
                        accum_out=acc[:],
                    )
                    accs.append(acc)
                ssum = small.tile([1, 1], f32, tag="ssum")
                nc.vector.tensor_add(ssum[:], accs[0][:], accs[1][:])
                rz = small.tile([1, 1], f32, tag="rz")
                nc.vector.reciprocal(rz[:], ssum[:])
                u_n = rows.tile([1, T], f32r, tag="un")
                nc.vector.tensor_scalar_mul(u_n[:], u_row[:], rz[0:1, 0:1])
                prev = {"xT": xT, "u_n": u_n, "it": it}
                if xT_next is not None:
                    xT = xT_next
            emit_finish(prev)

    nc.compile()
    return nc


def _get_nc(bl=BL):
    if bl not in _CACHE:
        _CACHE[bl] = build(bl)
    return _CACHE[bl]


def kernel(**inputs):
    import ml_dtypes
    from concourse.bass_utils import run_bass_kernel_spmd

    bf = ml_dtypes.bfloat16
    # host-side marshaling: bf16 conversion + pre-transposition (x -> [B,H,T],
    # W_enc/W_dec -> W^T[h,o]) so the device never transposes anything
    x = np.ascontiguousarray(
        np.asarray(inputs["spatial_feats"], dtype=np.float32)
        .astype(bf)
        .transpose(0, 2, 1)
    )
    hs = np.asarray(inputs["hidden_state"], dtype=np.float32).astype(bf)
    shared = {
        "W_enc": np.ascontiguousarray(
            np.asarray(inputs["W_enc"], dtype=np.float32).astype(bf).T
        ),
        "W_dec": np.ascontiguousarray(
            np.asarray(inputs["W_dec"], dtype=np.float32).astype(bf).T
        ),
        "w_score": np.ascontiguousarray(
            np.asarray(inputs["w_score"], dtype=np.float32).astype(bf)
        ),
        "b_enc": np.ascontiguousarray(np.asarray(inputs["b_enc"], dtype=np.float32)),
        "b_dec": np.ascontiguousarray(np.asarray(inputs["b_dec"], dtype=np.float32)),
    }
    nc = _get_nc()
    in_maps = []
    for i in range(NCORES):
        m = {
            "spatial_feats": x[i * BL : (i + 1) * BL],
            "hidden_state": np.ascontiguousarray(hs[i * BL : (i + 1) * BL]),
        }
        m.update(shared)
        in_maps.append(m)
    res = run_bass_kernel_spmd(nc, in_maps, core_ids=list(range(NCORES)))
    global LAST_RESULT
    LAST_RESULT = res
    ctx = np.concatenate(
        [
            res.results[i]["out_ctx"].transpose(0, 2, 1).reshape(BL, H)
            for i in range(NCORES)
        ],
        axis=0,
    )
    w = np.concatenate([res.results[i]["out_w"] for i in range(NCORES)], axis=0)
    return (ctx, w)


# revision 12
# speedup vs baseline: 1.4210x; 1.2514x over previous
"""Bahdanau-attention kernel for Trainium2, data-parallel over 8 NeuronCores.

V4: bf16 datapath, all tensors host-pre-transposed/laid-out so the device
does zero transposes and only a handful of coarse contiguous HWDGE DMAs.
GpSimd (Pool) is never used — its software sequencer adds ~2us per
semaphore op and poisons cross-engine dependency latency.

Per core (B_local=8, T=1024, H=1024), per batch b:
  eT[o,t]   = tanh(sum_h W_enc[o,h] x[t,h] + (W_dec h + b_dec + b_enc)[o])
              (PE bf16 matmuls from xT/wT; ScalarE tanh w/ per-partition bias)
  scores[t] = sum_o w_score[o] * eT[o,t]        (PE, ws chunk stationary)
  u         = exp(scores); w = u / sum(u)       (ScalarE exp+accum, DVE recip)
  u_rep     = ones^T w                          (PE K=1 matmul -> [128,T])
  ctxT[p,j] = sum_t xT[j][p,t] * u_rep[p,t]     (DVE fused scalar_tensor_tensor)
b_score dropped: softmax is shift-invariant so it cancels in both outputs.
Outputs: out_w [bl,T] rows; out_ctx [bl,P,NH] column-major (host transposes).
"""

import sys

if "/opt/trn_rl_repo" not in sys.path:
    sys.path.insert(0, "/opt/trn_rl_repo")

import numpy as np

B, T, H = 64, 1024, 1024
NCORES = 8
BL = B // NCORES  # batches per core
P = 128  # partitions
NH = H // P  # h chunks
NO = H // P  # o chunks
NS = 2  # free-dim halves of T
F = 512  # matmul free-dim slice (one PSUM bank of f32)

_CACHE = {}
LAST_RESULT = None


def build(bl=BL):
    import concourse.tile as tile
    from concourse import bacc, mybir

    f32 = mybir.dt.float32
    f32r = mybir.dt.float32r
    bf16 = mybir.dt.bfloat16
    AF = mybir.ActivationFunctionType
    OP = mybir.AluOpType

    nc = bacc.Bacc("TRN2", target_bir_lowering=False, debug=False, num_devices=NCORES)
    # host-prepared layouts:
    #   x_d[b, h, t]        (x transposed per batch)
    #   we_d[h, o], wd_d[h, o]  (W^T)
    #   hst_d[p, c*bl+b] = hidden_state[b, c*P+p]
    #   bsum_d[p, c] = (b_enc + b_dec)[c*P+p];  ws_d[p, c] = w_score[c*P+p]
    x_d = nc.declare_dram_parameter("spatial_feats", [bl, H, T], bf16, isOutput=False)
    hst_d = nc.declare_dram_parameter("hsT", [P, NH * bl], bf16, isOutput=False)
    we_d = nc.declare_dram_parameter("W_enc", [H, H], bf16, isOutput=False)
    wd_d = nc.declare_dram_parameter("W_dec", [H, H], bf16, isOutput=False)
    bsum_d = nc.declare_dram_parameter("bsum", [P, NH], f32, isOutput=False)
    ws_d = nc.declare_dram_parameter("w_score", [P, NH], bf16, isOutput=False)
    ctx_d = nc.declare_dram_parameter("out_ctx", [bl, P, NH], f32, isOutput=True)
    wout_d = nc.declare_dram_parameter("out_w", [bl, T], f32, isOutput=True)

    with tile.TileContext(nc) as tc:
        with (
            tc.tile_pool(name="const", bufs=1) as const,
            tc.tile_pool(name="wt", bufs=1) as wt_pool,
            tc.tile_pool(name="wdt", bufs=1) as wdt_pool,
            tc.tile_pool(name="xt", bufs=3) as xt_pool,
            tc.tile_pool(name="et", bufs=6) as et_pool,
            tc.tile_pool(name="urep", bufs=2) as urep_pool,
            tc.tile_pool(name="ctxsc", bufs=2) as ctxsc_pool,
            tc.tile_pool(name="ctxt", bufs=2) as ctxt_pool,
            tc.tile_pool(name="rows", bufs=4) as rows,
            tc.tile_pool(name="small", bufs=16) as small,
            tc.tile_pool(name="mmps", bufs=4, space="PSUM") as mm_ps,
            tc.tile_pool(name="scps", bufs=2, space="PSUM") as sc_ps,
            tc.tile_pool(name="urps", bufs=2, space="PSUM") as ur_ps,
        ):
            # ---- constants / small loads (scalar queue) ----
            ones_f = const.tile([1, P], f32, tag="onesf")
            nc.vector.memset(ones_f[:], 1.0)
            ones_col = const.tile([1, P], f32r, tag="ones")
            nc.vector.tensor_copy(ones_col[:], ones_f[:])

            bsum = const.tile([P, NH], f32, tag="bsum")
            nc.scalar.dma_start(bsum[:], bsum_d[:])
            ws_sb = const.tile([P, NH], bf16, tag="ws")
            nc.scalar.dma_start(ws_sb[:], ws_d[:])
            hsT = const.tile([P, NH * bl], bf16, tag="hsT")
            nc.scalar.dma_start(hsT[:], hst_d[:])

            # ---- W_dec^T as one coarse DMA (sync queue, first: PE bias
            # matmuls are queued before enc and must not stall late) ----
            wdT = wdt_pool.tile([P, NH * H], bf16, tag="wdt")
            nc.sync.dma_start(
                wdT[:].rearrange("p (j o) -> p j o", j=NH),
                wd_d[:].rearrange("(j p) o -> p j o", p=P),
            )

            # ---- bias_all[o_off, ot*bl+b] = (W_dec h_b + b_dec + b_enc)[o] ----
            bias_all = const.tile([P, NO * bl], f32, tag="bias_all")
            for ot in range(NO):
                psd = mm_ps.tile([P, F], f32, tag="mmps")
                for j in range(NH):
                    nc.tensor.matmul(
                        psd[:, 0:bl],
                        wdT[:, j * H + ot * P : j * H + (ot + 1) * P],
                        hsT[:, j * bl : (j + 1) * bl],
                        start=(j == 0),
                        stop=(j == NH - 1),
                    )
                nc.vector.tensor_scalar_add(
                    bias_all[:, ot * bl : (ot + 1) * bl],
                    psd[:, 0:bl],
                    bsum[:, ot : ot + 1],
                )

            # ---- W_enc^T and x(0)^T: coarse DMAs in halves for pacing ----
            wT = wt_pool.tile([P, NH * H], bf16, tag="wt")
            xT0 = xt_pool.tile([P, NH * T], bf16, tag="xt", name="xT0")
            half = NH // 2
            for hf in range(2):
                r0, r1 = hf * half * P, (hf + 1) * half * P
                nc.sync.dma_start(
                    wT[:, hf * half * H : (hf + 1) * half * H].rearrange(
                        "p (j o) -> p j o", j=half
                    ),
                    we_d[r0:r1, :].rearrange("(j p) o -> p j o", p=P),
                )
                nc.scalar.dma_start(
                    xT0[:, hf * half * T : (hf + 1) * half * T].rearrange(
                        "p (j t) -> p j t", j=half
                    ),
                    x_d[0, r0:r1, :].rearrange("(j p) t -> p j t", p=P),
                )

            def emit_enc_group(xT, it, o, s):
                ps = mm_ps.tile([P, F], f32, tag="mmps")
                for h in range(NH):
                    nc.tensor.matmul(
                        ps[:],
                        wT[:, h * H + o * P : h * H + (o + 1) * P],
                        xT[:, h * T + s * F : h * T + (s + 1) * F],
                        start=(h == 0),
                        stop=(h == NH - 1),
                    )
                e = et_pool.tile([P, F], bf16, tag="et")
                nc.scalar.activation(
                    e[:],
                    ps[:],
                    AF.Tanh,
                    bias=bias_all[:, o * bl + it : o * bl + it + 1],
                    scale=1.0,
                )
                return e

            def emit_finish(st):
                """Batch-tail work for batch st['it']: replicate normalized
                weights across partitions, fused multiply-reduce context,
                DMA both outputs."""
                it = st["it"]
                u_n = st["u_n"]
                urp = []
                for s in range(NS):
                    pu = ur_ps.tile([P, F], f32, tag="urps")
                    nc.tensor.matmul(
                        pu[:],
                        ones_col[:],
                        u_n[0:1, s * F : (s + 1) * F],
                        start=True,
                        stop=True,
                    )
                    urp.append(pu)
                u_rep = urep_pool.tile([P, T], bf16, tag="urep")
                for s in range(NS):
                    nc.scalar.activation(
                        u_rep[:, s * F : (s + 1) * F], urp[s][:], AF.Copy
                    )
                ctxT = ctxt_pool.tile([P, NH], f32, tag="ctxt")
                for j in range(NH):
                    scr = ctxsc_pool.tile([P, T], bf16, tag="ctxsc")
                    nc.vector.scalar_tensor_tensor(
                        out=scr[:],
                        in0=st["xT"][:, j * T : (j + 1) * T],
                        scalar=1.0,
                        in1=u_rep[:],
                        op0=OP.mult,
                        op1=OP.mult,
                        accum_out=ctxT[:, j : j + 1],
                    )
                nc.sync.dma_start(ctx_d[it], ctxT[:])
                nc.sync.dma_start(wout_d[it : it + 1, :], u_n[:].bitcast(f32))

            # ---- main per-batch pipeline ----
            prev = None
            xT = xT0
            for it in range(bl):
                eT = {}
                pss = None
                xT_next = None
                for o in range(NO):
                    for s in range(NS):
                        eT[(o, s)] = emit_enc_group(xT, it, o, s)
                    if o == 0:
                        # After the first enc group: finish batch it-1 (its
                        # softmax chain has completed during this group), then
                        # prefetch x(it+1) as one coarse DMA.
                        if prev is not None:
                            emit_finish(prev)
                            prev = None
                        if it + 1 < bl:
                            xT_next = xt_pool.tile(
                                [P, NH * T], bf16, tag="xt", name=f"xT{it + 1}"
                            )
                            nc.sync.dma_start(
                                xT_next[:].rearrange("p (j t) -> p j t", j=NH),
                                x_d[it + 1].rearrange("(j p) t -> p j t", p=P),
                            )
                        pss = {
                            s: sc_ps.tile([1, F], f32, tag="scps", name=f"pss{s}")
                            for s in range(NS)
                        }
                    else:
                        # scores run one o-chunk behind enc so the PE never
                        # waits on ScalarE's tanh
                        for s in range(NS):
                            nc.tensor.matmul(
                                pss[s][:],
                                ws_sb[:, o - 1 : o],
                                eT[(o - 1, s)][:],
                                start=(o - 1 == 0),
                                stop=False,
                            )
                # last scores group + softmax head (row layout)
                u_row = rows.tile([1, T], f32, tag="urow")
                accs = []
                for s in range(NS):
                    nc.tensor.matmul(
                        pss[s][:],
                        ws_sb[:, NO - 1 : NO],
                        eT[(NO - 1, s)][:],
                        start=False,
                        stop=True,
                    )
                    acc = small.tile([1, 1], f32, tag="acc")
                    nc.scalar.activation(
                        u_row[0:1, s * F : (s + 1) * F],
                        pss[s][:],
                        AF.Exp,
                        bias=0.0,
                        scale=1.0,
                        accum_out=acc[:],
                    )
                    accs.append(acc)
                ssum = small.tile([1, 1], f32, tag="ssum")
                nc.vector.tensor_add(ssum[:], accs[0][:], accs[1][:])
                rz = small.tile([1, 1], f32, tag="rz")
                nc.vector.reciprocal(rz[:], ssum[:])
                u_n = rows.tile([1, T], f32r, tag="un")
                nc.vector.tensor_scalar_mul(u_n[:], u_row[:], rz[0:1, 0:1])
                prev = {"xT": xT, "u_n": u_n, "it": it}
                if xT_next is not None:
                    xT = xT_next
            emit_finish(prev)

    nc.compile()
    return nc


def _get_nc(bl=BL):
    if bl not in _CACHE:
        _CACHE[bl] = build(bl)
    return _CACHE[bl]


def kernel(**inputs):
    import ml_dtypes
    from concourse.bass_utils import run_bass_kernel_spmd

    bf = ml_dtypes.bfloat16
    # host-side marshaling: bf16 conversion + all layout prep (transposes,
    # bias/score/hidden-state relayouts) so the device does zero transposes
    x = np.ascontiguousarray(
        np.asarray(inputs["spatial_feats"], dtype=np.float32)
        .astype(bf)
        .transpose(0, 2, 1)
    )
    hs = np.asarray(inputs["hidden_state"], dtype=np.float32).astype(bf)
    bsum = (
        np.asarray(inputs["b_enc"], dtype=np.float32)
        + np.asarray(inputs["b_dec"], dtype=np.float32)
    ).reshape(NH, P).T
    shared = {
        "W_enc": np.ascontiguousarray(
            np.asarray(inputs["W_enc"], dtype=np.float32).astype(bf).T
        ),
        "W_dec": np.ascontiguousarray(
            np.asarray(inputs["W_dec"], dtype=np.float32).astype(bf).T
        ),
        "bsum": np.ascontiguousarray(bsum),
        "w_score": np.ascontiguousarray(
            np.asarray(inputs["w_score"], dtype=np.float32).astype(bf).reshape(NH, P).T
        ),
    }
    nc = _get_nc()
    in_maps = []
    for i in range(NCORES):
        hs_slice = hs[i * BL : (i + 1) * BL]
        hsT = np.ascontiguousarray(
            hs_slice.reshape(BL, NH, P).transpose(2, 1, 0).reshape(P, NH * BL)
        )
        m = {
            "spatial_feats": x[i * BL : (i + 1) * BL],
            "hsT": hsT,
        }
        m.update(shared)
        in_maps.append(m)
    res = run_bass_kernel_spmd(nc, in_maps, core_ids=list(range(NCORES)))
    global LAST_RESULT
    LAST_RESULT = res
    ctx = np.concatenate(
        [
            res.results[i]["out_ctx"].transpose(0, 2, 1).reshape(BL, H)
            for i in range(NCORES)
        ],
        axis=0,
    )
    w = np.concatenate([res.results[i]["out_w"] for i in range(NCORES)], axis=0)
    return (ctx, w)
